# revision 14
# baseline (speedup 1.0000x reference)
"""Trainium2 Bass kernel for Llama attention (B=2, S=2048, H=4096, 32 heads).

Sharding: tensor-parallel across heads over 8 NeuronCores. Each core owns
d_shard = 512 hidden dims (4 heads): Wq/Wk/Wv column-sharded, Wo row-sharded.
All cores see the full (transposed) hidden states; partial outputs are summed
on the host (the Wo row-parallel all-reduce).

Per-core device program (one NEFF, three phases through DRAM intermediates):
  A) QKV projections. Q^T,K^T produced in [d, tok] layout (head dim on
     partitions) with RoPE fused into the PSUM evacuation; V in [tok, d].
  B) Causal attention per (batch, head), done entirely in the transposed
     score layout S^T[k, q] so softmax needs no transposes:
       exp (no max-subtraction; scores are O(1) by construction),
       block-skip + triangular masks for causality,
       row-sums via a ones-vector matmul, 1/L via Log/Exp on ScalarE,
       unnormalized O^T = V^T-accumulation, normalized on PSUM evacuation.
  C) out = O^T.T @ Wo accumulated over the 4 local heads.
All matmuls run as float32r (full-rate fp32 on the PE at N>=256).
"""

import math
import os
import sys

import numpy as np

for _p in ("/root/.axon_site/_ro/trn_rl_repo", "/opt/trn_rl_repo"):
    if os.path.isdir(_p) and _p not in sys.path:
        sys.path.append(_p)

import concourse.bass as bass
import concourse.mybir as mybir
import concourse.tile as tile
from concourse import bacc
from concourse import bass_utils

# Bind Exp AND Ln to the one table set containing both
# (natural_log_exp_and_others). The default chooser binds Exp to
# exp_and_others and Ln to natural_log, which makes the ACT stream reload
# table sets (~2.7us each) around every softmax-denominator Ln. Indices into
# act_info.json must be preserved, so only set CONTENTS are edited.
_orig_get_act_tables = bacc.get_activation_tables


def _patched_get_act_tables(arch):
    tabs = {k: set(v) for k, v in _orig_get_act_tables(arch).items()}
    AF = mybir.ActivationFunctionType
    if "natural_log_exp_and_others" in tabs:
        combined = tabs["natural_log_exp_and_others"]
        if AF.Exp in combined and AF.Ln in combined:
            for name, fns in tabs.items():
                if name != "natural_log_exp_and_others":
                    fns.discard(AF.Exp)
                    fns.discard(AF.Ln)
    return tabs


bacc.get_activation_tables = _patched_get_act_tables

F32 = mybir.dt.float32
F32R = mybir.dt.float32r
BF16 = mybir.dt.bfloat16

HIDDEN = 4096
NUM_HEADS = 32
HEAD_DIM = 128
ROPE_BASE = 10000.0
N_CORES = 8


class Cfg:
    def __init__(self, hidden=HIDDEN, d_shard=HIDDEN // N_CORES, s_batch=2048,
                 n_batch=2, tokt=512, tgrp=2, bf16a=False):
        self.bf16a = bf16a
        self.hidden = hidden
        self.d_shard = d_shard
        self.s_batch = s_batch
        self.n_batch = n_batch
        self.tokt = tokt          # token tile (psum free dim)
        self.tgrp = tgrp          # token tiles per phase-A group
        self.KC = hidden // 128   # contraction chunks
        self.HL = d_shard // HEAD_DIM   # local heads
        self.NTOK = n_batch * s_batch
        self.NT = self.NTOK // tokt
        assert self.NT % tgrp == 0
        self.NG = self.NT // tgrp
        assert s_batch % tokt == 0  # a token tile never straddles batches
        self.KCPB = s_batch // 128      # key chunks per batch
        self.QTPB = s_batch // tokt     # q tiles per batch
        self.NDIAG = tokt // 128        # diagonal 128-blocks per q tile
        self.scale = HEAD_DIM ** -0.5


def build_nc(cfg: Cfg, n_cores=N_CORES, phases="ABC"):
    c = cfg
    nc = bacc.Bacc("TRN2", target_bir_lowering=False, debug=False,
                   num_devices=n_cores)
    a_dt = BF16 if c.bf16a else F32
    hsT = nc.dram_tensor("hsT", [c.hidden, c.NTOK], a_dt, kind="ExternalInput")
    Wq = nc.dram_tensor("Wq", [c.hidden, c.d_shard], a_dt, kind="ExternalInput")
    Wk = nc.dram_tensor("Wk", [c.hidden, c.d_shard], a_dt, kind="ExternalInput")
    Wv = nc.dram_tensor("Wv", [c.hidden, c.d_shard], a_dt, kind="ExternalInput")
    Wo = nc.dram_tensor("Wo", [c.d_shard, c.hidden], F32, kind="ExternalInput")
    cosT = nc.dram_tensor("cosT", [128, c.s_batch], F32, kind="ExternalInput")
    sinT = nc.dram_tensor("sinT", [128, c.s_batch], F32, kind="ExternalInput")
    out = nc.dram_tensor("out", [c.NTOK, c.hidden], F32, kind="ExternalOutput")
    QTd = nc.dram_tensor("QTd", [c.d_shard, c.NTOK], F32, kind="Internal")
    KTd = nc.dram_tensor("KTd", [c.d_shard, c.NTOK], F32, kind="Internal")
    Vd = nc.dram_tensor("Vd", [c.NTOK, c.d_shard], F32, kind="Internal")
    OTd = nc.dram_tensor("OTd", [c.d_shard, c.NTOK], F32, kind="Internal")

    AF = mybir.ActivationFunctionType
    with tile.TileContext(nc) as tc:
        if True:
            # ------------- Phase A2 (bf16): W resident, hs streamed -------------
            if "A" in phases and c.bf16a:
              with (tc.tile_pool(name="constA", bufs=1) as cpool,
                    tc.tile_pool(name="wres", bufs=1) as wrp,
                    tc.tile_pool(name="hs", bufs=2) as hsp,
                    tc.tile_pool(name="evA", bufs=2) as evp,
                    tc.tile_pool(name="psA", bufs=2, space="PSUM") as psA):
                cos_sb = cpool.tile([128, c.s_batch], F32, tag="cos")
                nc.sync.dma_start(cos_sb[:], cosT.ap())
                sin_sb = cpool.tile([128, c.s_batch], F32, tag="sin")
                nc.sync.dma_start(sin_sb[:], sinT.ap())
                wq_res, wk_res = [], []
                for wdram, lst in ((Wq, wq_res), (Wk, wk_res)):
                    for h in range(c.HL):
                        wt = wrp.tile([128, c.KC * 128], BF16,
                                      tag=f"w{wdram.name}{h}",
                                      name=f"wres_{wdram.name}{h}")
                        nc.sync.dma_start(
                            wt[:], wdram.ap()[:, h * 128:(h + 1) * 128]
                            .rearrange("(c p) n -> p c n", p=128))
                        lst.append(wt)
                wv_res = []
                for k in range(c.KC):
                    wt = wrp.tile([128, c.d_shard], BF16, tag=f"wv{k}",
                                  name=f"wres_v{k}")
                    nc.sync.dma_start(wt[:], Wv.ap()[k * 128:(k + 1) * 128, :])
                    wv_res.append(wt)
                for n in range(c.NT):
                    col0 = n * c.tokt
                    lc = col0 % c.s_batch
                    hts = []
                    for k in range(c.KC):
                        t = hsp.tile([128, c.tokt], BF16, tag=f"hs{k}")
                        nc.sync.dma_start(
                            t[:], hsT.ap()[k * 128:(k + 1) * 128,
                                           col0:col0 + c.tokt])
                        hts.append(t)
                    for wres, outdram in ((wq_res, QTd), (wk_res, KTd)):
                        for h in range(c.HL):
                            ps = psA.tile([128, c.tokt], F32, tag="psA")
                            for k in range(c.KC):
                                nc.tensor.matmul(
                                    ps[:], wres[h][:, k * 128:(k + 1) * 128],
                                    hts[k][:], start=(k == 0),
                                    stop=(k == c.KC - 1))
                            a = evp.tile([128, c.tokt], F32, tag="ev_a")
                            bt = evp.tile([128, c.tokt], F32, tag="ev_b")
                            o = evp.tile([128, c.tokt], F32, tag="ev_o")
                            nc.vector.tensor_mul(
                                a[:], ps[:], cos_sb[:, lc:lc + c.tokt])
                            nc.vector.tensor_mul(
                                bt[0:64, :], ps[64:128, :],
                                sin_sb[0:64, lc:lc + c.tokt])
                            nc.vector.tensor_mul(
                                bt[64:128, :], ps[0:64, :],
                                sin_sb[64:128, lc:lc + c.tokt])
                            nc.vector.tensor_add(o[:], a[:], bt[:])
                            nc.sync.dma_start(
                                outdram.ap()[h * 128:(h + 1) * 128,
                                             col0:col0 + c.tokt], o[:])
                    for ts in range(c.tokt // 128):
                        ps = psA.tile([128, c.d_shard], F32, tag="psA")
                        for k in range(c.KC):
                            nc.tensor.matmul(
                                ps[:], hts[k][:, ts * 128:(ts + 1) * 128],
                                wv_res[k][:], start=(k == 0),
                                stop=(k == c.KC - 1))
                        ev = evp.tile([128, c.d_shard], F32, tag="ev_v")
                        nc.scalar.copy(ev[:], ps[:])
                        r0 = col0 + ts * 128
                        nc.sync.dma_start(Vd.ap()[r0:r0 + 128, :], ev[:])

            # ---------------- Phase A: projections + RoPE ----------------
            if "A" not in phases or c.bf16a:
                pass
            else:
              with (tc.tile_pool(name="constA", bufs=1) as cpool,
                  tc.tile_pool(name="hs", bufs=1) as hsp,
                  tc.tile_pool(name="wqk", bufs=2) as wp,
                  tc.tile_pool(name="wv", bufs=3) as wvp,
                  tc.tile_pool(name="evA", bufs=3) as evp,
                  tc.tile_pool(name="psA", bufs=3, space="PSUM") as psA,
                  tc.tile_pool(name="psV", bufs=1, space="PSUM") as psV):
                cos_sb = cpool.tile([128, c.s_batch], F32, tag="cos")
                nc.sync.dma_start(cos_sb[:], cosT.ap())
                sin_sb = cpool.tile([128, c.s_batch], F32, tag="sin")
                nc.sync.dma_start(sin_sb[:], sinT.ap())
                gw = c.tokt * c.tgrp
                for g in range(c.NG):
                    gcol = g * gw
                    hts = []
                    for k in range(c.KC):
                        t = hsp.tile([128, gw], F32R, tag=f"hs{k}")
                        nc.sync.dma_start(
                            t[:],
                            hsT.ap()[k * 128:(k + 1) * 128,
                                     gcol:gcol + gw].bitcast(F32R))
                        hts.append(t)
                    for wdram, outdram in ((Wq, QTd), (Wk, KTd)):
                        for h in range(c.HL):
                            wt = wp.tile([128, c.KC * 128], F32R, tag="w")
                            nc.sync.dma_start(
                                wt[:],
                                wdram.ap()[:, h * 128:(h + 1) * 128]
                                .rearrange("(c p) n -> p c n", p=128)
                                .bitcast(F32R))
                            for nl in range(c.tgrp):
                                ps = psA.tile([128, c.tokt], F32, tag="psA")
                                for k in range(c.KC):
                                    nc.tensor.matmul(
                                        ps[:],
                                        wt[:, k * 128:(k + 1) * 128],
                                        hts[k][:, nl * c.tokt:(nl + 1) * c.tokt],
                                        start=(k == 0), stop=(k == c.KC - 1))
                                col0 = gcol + nl * c.tokt
                                lc = col0 % c.s_batch
                                a = evp.tile([128, c.tokt], F32, tag="ev_a")
                                bt = evp.tile([128, c.tokt], F32, tag="ev_b")
                                o = evp.tile([128, c.tokt], F32, tag="ev_o")
                                nc.vector.tensor_mul(
                                    a[:], ps[:], cos_sb[:, lc:lc + c.tokt])
                                nc.vector.tensor_mul(
                                    bt[0:64, :], ps[64:128, :],
                                    sin_sb[0:64, lc:lc + c.tokt])
                                nc.vector.tensor_mul(
                                    bt[64:128, :], ps[0:64, :],
                                    sin_sb[64:128, lc:lc + c.tokt])
                                nc.vector.tensor_add(o[:], a[:], bt[:])
                                nc.sync.dma_start(
                                    outdram.ap()[h * 128:(h + 1) * 128,
                                                 col0:col0 + c.tokt], o[:])
                    # V = hs @ Wv in [tok, d] layout; k-outer over 4 psum banks
                    for nl in range(c.tgrp):
                        vps = [psV.tile([128, c.d_shard], F32, tag=f"v{i}",
                                        name=f"vps{g}_{nl}_{i}")
                               for i in range(c.tokt // 128)]
                        for k in range(c.KC):
                            wv = wvp.tile([128, c.d_shard], F32R, tag="wv")
                            nc.sync.dma_start(
                                wv[:],
                                Wv.ap()[k * 128:(k + 1) * 128, :].bitcast(F32R))
                            for i in range(c.tokt // 128):
                                t0 = nl * c.tokt + i * 128
                                nc.tensor.matmul(
                                    vps[i][:], hts[k][:, t0:t0 + 128], wv[:],
                                    start=(k == 0), stop=(k == c.KC - 1))
                        for i in range(c.tokt // 128):
                            ev = evp.tile([128, c.d_shard], F32, tag="ev_v")
                            nc.scalar.copy(ev[:], vps[i][:])
                            r0 = gcol + nl * c.tokt + i * 128
                            nc.sync.dma_start(Vd.ap()[r0:r0 + 128, :], ev[:])

            # ---------------- Phase B: causal attention ----------------
            if "B" not in phases:
                pass
            else:
              with (tc.tile_pool(name="constB", bufs=1) as cbp,
                  tc.tile_pool(name="qkv", bufs=3) as qkvp,
                  tc.tile_pool(name="pb", bufs=3) as pbp,
                  tc.tile_pool(name="sm", bufs=2) as smp,
                  tc.tile_pool(name="psS", bufs=4, space="PSUM") as psS,
                  tc.tile_pool(name="psO", bufs=2, space="PSUM") as psO,
                  tc.tile_pool(name="psL", bufs=2, space="PSUM") as psL):
                ones_f = cbp.tile([128, 1], F32, tag="ones_f")
                nc.vector.memset(ones_f[:], 1.0)
                ones_sb = cbp.tile([128, 1], F32R, tag="ones")
                nc.vector.tensor_copy(ones_sb[:], ones_f[:])
                masks = []
                for t in range(c.NDIAG):
                    mf = cbp.tile([128, c.tokt], F32, tag=f"maskf{t}")
                    nc.gpsimd.memset(mf[:], 1.0)
                    # keep where q - k - 128*t >= 0 (q free, k partition)
                    nc.gpsimd.affine_select(
                        out=mf[:], in_=mf[:], compare_op=mybir.AluOpType.is_ge,
                        fill=0.0, base=-(128 * t), pattern=[[1, c.tokt]],
                        channel_multiplier=-1)
                    m = cbp.tile([128, c.tokt], F32R, tag=f"mask{t}")
                    nc.vector.tensor_copy(m[:], mf[:])
                    masks.append(m)
                for b in range(c.n_batch):
                    for h in range(c.HL):
                        s0 = b * c.s_batch
                        qt = qkvp.tile([128, c.s_batch], F32R, tag="qt")
                        nc.sync.dma_start(
                            qt[:], QTd.ap()[h * 128:(h + 1) * 128,
                                            s0:s0 + c.s_batch].bitcast(F32R))
                        kt = qkvp.tile([128, c.s_batch], F32R, tag="kt")
                        nc.sync.dma_start(
                            kt[:], KTd.ap()[h * 128:(h + 1) * 128,
                                            s0:s0 + c.s_batch].bitcast(F32R))
                        vt = qkvp.tile([128, c.KCPB * 128], F32R, tag="vt")
                        nc.sync.dma_start(
                            vt[:],
                            Vd.ap()[s0:s0 + c.s_batch, h * 128:(h + 1) * 128]
                            .rearrange("(c p) n -> p c n", p=128)
                            .bitcast(F32R))
                        for j in range(c.QTPB):
                            nchunks = (j + 1) * c.tokt // 128
                            ot_ps = psO.tile([128, c.tokt], F32, tag="ot")
                            l_ps = psL.tile([1, c.tokt], F32, tag="l")
                            prev = None

                            def flush(last):
                                p_, i_ = prev
                                nc.tensor.matmul(
                                    ot_ps[:], vt[:, i_ * 128:(i_ + 1) * 128],
                                    p_[:], start=(i_ == 0), stop=last)
                                nc.tensor.matmul(
                                    l_ps[:], ones_sb[:], p_[:],
                                    start=(i_ == 0), stop=last)

                            for i in range(nchunks):
                                s_ps = psS.tile([128, c.tokt], F32, tag="s")
                                nc.tensor.matmul(
                                    s_ps[:], kt[:, i * 128:(i + 1) * 128],
                                    qt[:, j * c.tokt:(j + 1) * c.tokt],
                                    start=True, stop=True)
                                p = pbp.tile([128, c.tokt], F32R, tag="p")
                                nc.scalar.activation(p[:], s_ps[:], AF.Exp,
                                                     scale=c.scale)
                                td = i - (j * c.tokt) // 128
                                if td >= 0:
                                    nc.vector.tensor_mul(p[:], p[:],
                                                         masks[td][:])
                                if prev is not None:
                                    flush(False)
                                prev = (p, i)
                            flush(True)
                            lg = smp.tile([1, c.tokt], F32, tag="lg")
                            nc.scalar.activation(lg[:], l_ps[:], AF.Ln)
                            rc = smp.tile([1, c.tokt], F32, tag="rc")
                            nc.scalar.activation(rc[:], lg[:], AF.Exp,
                                                 scale=-1.0)
                            bc = smp.tile([128, c.tokt], F32, tag="bc")
                            nc.gpsimd.partition_broadcast(bc[:], rc[:])
                            q0 = s0 + j * c.tokt
                            otn = smp.tile([128, c.tokt], F32, tag="otn",
                                           bufs=3)
                            nc.vector.tensor_mul(otn[:], ot_ps[:], bc[:])
                            nc.sync.dma_start(
                                OTd.ap()[h * 128:(h + 1) * 128,
                                         q0:q0 + c.tokt], otn[:])

            # ---------------- Phase C: output projection ----------------
            if "C" not in phases:
                pass
            else:
              with (tc.tile_pool(name="wo", bufs=1) as wop,
                  tc.tile_pool(name="otm", bufs=2) as otmp,
                  tc.tile_pool(name="evC", bufs=6) as evc,
                  tc.tile_pool(name="psC", bufs=6, space="PSUM") as psC):
                n_ct = c.hidden // c.tokt
                n_mt = c.NTOK // 128
                wts = []
                for jd in range(c.HL):
                    wt = wop.tile([128, c.hidden], F32R, tag=f"wo{jd}")
                    nc.sync.dma_start(
                        wt[:], Wo.ap()[jd * 128:(jd + 1) * 128, :]
                        .bitcast(F32R))
                    wts.append(wt)
                for m in range(n_mt):
                    oms = []
                    for jd in range(c.HL):
                        om = otmp.tile([128, 128], F32R, tag=f"otm{jd}")
                        nc.sync.dma_start(
                            om[:], OTd.ap()[jd * 128:(jd + 1) * 128,
                                            m * 128:(m + 1) * 128]
                            .bitcast(F32R))
                        oms.append(om)
                    for ci in range(n_ct):
                        ps = psC.tile([128, c.tokt], F32, tag="c")
                        for jd in range(c.HL):
                            nc.tensor.matmul(
                                ps[:], oms[jd][:],
                                wts[jd][:, ci * c.tokt:(ci + 1) * c.tokt],
                                start=(jd == 0), stop=(jd == c.HL - 1))
                        ev = evc.tile([128, c.tokt], F32, tag="ev")
                        if ci % 2 == 0:
                            nc.vector.tensor_copy(ev[:], ps[:])
                        else:
                            nc.scalar.copy(ev[:], ps[:])
                        nc.sync.dma_start(
                            out.ap()[m * 128:(m + 1) * 128,
                                     ci * c.tokt:(ci + 1) * c.tokt], ev[:])
    nc.compile()
    return nc


def rope_tables(positions, s_batch):
    pos = np.asarray(positions).astype(np.float64)
    inv = ROPE_BASE ** (-np.arange(0, HEAD_DIM, 2, dtype=np.float64) / HEAD_DIM)
    fr = pos[None, :] * inv[:, None]            # [64, S]
    cosT = np.concatenate([np.cos(fr), np.cos(fr)], 0).astype(np.float32)
    sinT = np.concatenate([-np.sin(fr), np.sin(fr)], 0).astype(np.float32)
    return np.ascontiguousarray(cosT), np.ascontiguousarray(sinT)


def make_in_maps(cfg, positions, hidden_states, Wq, Wk, Wv, Wo, n_cores=N_CORES):
    c = cfg
    if c.bf16a:
        import ml_dtypes
        a_np = ml_dtypes.bfloat16
    else:
        a_np = np.float32
    hs = np.asarray(hidden_states, dtype=np.float32)
    hsT = np.ascontiguousarray(hs.reshape(c.NTOK, c.hidden).T.astype(a_np))
    cosT, sinT = rope_tables(positions, c.s_batch)
    Wq = np.asarray(Wq, dtype=np.float32).astype(a_np)
    Wk = np.asarray(Wk, dtype=np.float32).astype(a_np)
    Wv = np.asarray(Wv, dtype=np.float32).astype(a_np)
    Wo = np.asarray(Wo, dtype=np.float32)
    in_maps = []
    for r in range(n_cores):
        d0 = r * c.d_shard
        in_maps.append({
            "hsT": hsT,
            "Wq": np.ascontiguousarray(Wq[:, d0:d0 + c.d_shard]),
            "Wk": np.ascontiguousarray(Wk[:, d0:d0 + c.d_shard]),
            "Wv": np.ascontiguousarray(Wv[:, d0:d0 + c.d_shard]),
            "Wo": np.ascontiguousarray(Wo[d0:d0 + c.d_shard, :]),
            "cosT": cosT,
            "sinT": sinT,
        })
    return in_maps


_NC_CACHE = {}


def get_nc(cfg=None, n_cores=N_CORES):
    cfg = cfg or Cfg()
    key = (cfg.hidden, cfg.d_shard, cfg.s_batch, cfg.n_batch, n_cores, cfg.bf16a)
    if key not in _NC_CACHE:
        _NC_CACHE[key] = (cfg, build_nc(cfg, n_cores))
    return _NC_CACHE[key]


def kernel(positions, hidden_states, Wq, Wk, Wv, Wo):
    cfg, nc = get_nc()
    in_maps = make_in_maps(cfg, positions, hidden_states, Wq, Wk, Wv, Wo)
    res = bass_utils.run_bass_kernel_spmd(nc, in_maps,
                                          core_ids=list(range(N_CORES)))
    acc = np.zeros((cfg.NTOK, cfg.hidden), dtype=np.float32)
    for r in res.results:
        acc += r["out"]
    return acc.reshape(cfg.n_batch, cfg.s_batch, cfg.hidden)
